# revision 5
# baseline (speedup 1.0000x reference)
# VQ codebook (soft k-means) kernel for Trainium2, 8 NeuronCores SPMD.
#
# Math (reference): e = l2norm(x rows); repeat num_iter+1 times:
#   dist = e @ l2norm(mu).T ; r = softmax(beta*dist, axis=1)
#   mu <- (r.T @ e) / r.sum(0)[:, None]
# then final r = softmax(beta * e @ mu.T) with UN-normalized mu.
# Returns (mu, r).
#
# Distribution: rows of x (n axis) sharded over 8 cores; mu replicated;
# the (k, d+1) partial sums [r.T @ e | r.sum(0)] AllReduce'd each pass.
#
# Device layout per core (n_local = 32768 rows, 256 row-tiles of 128):
#   eT   (128=d, n_local) bf16   - lhsT for dist matmuls (host pre-transposed)
#   eaug (n_local, 129)   bf16   - [e | 1] rows, rhs for cluster-mean matmuls
#   dist tile = matmul(lhsT=eT_tile, rhs=munT) -> psum (128=n, 256=k)
#   s = exp(beta*dist) on ACT; Z = row-sums via DVE add-tree; on GPSIMD
#   ehat = eaug_tile * (1/Z) then psum_cm[k_half,129] += s_half.T @ ehat on PE.

import numpy as np
import ml_dtypes

import concourse.bass as bass
import concourse.mybir as mybir
import concourse.tile as tile
from concourse import bacc
from concourse.bass_utils import run_bass_kernel_spmd
from concourse.masks import make_identity

N_CORES = 8
N, D, K = 262144, 128, 256
NL = N // N_CORES          # 32768 rows per core
T = NL // 128              # 256 row-tiles per core
BETA = 5.0
FP = mybir.dt.float32
BF = mybir.dt.bfloat16
AF = mybir.ActivationFunctionType
CH_U = 4                   # row-tiles per chunk in mu-update passes
CH_F = 8                   # row-tiles per chunk in final pass

_cache = {}


def _zree(nc, sb, s, nt, Z):
    """Row-sums of s (128, nt, 256) -> Z (128, nt) fp32 via bf16 add tree."""
    h1 = sb.tile([128, nt * 128], BF, tag="zh1")
    s3 = s[:].rearrange("p (j k) -> p j k", j=nt)
    h13 = h1[:].rearrange("p (j k) -> p j k", j=nt)
    nc.vector.tensor_add(h13, s3[:, :, 0:128], s3[:, :, 128:256])
    h2 = sb.tile([128, nt * 64], BF, tag="zh2")
    h23 = h2[:].rearrange("p (j k) -> p j k", j=nt)
    nc.vector.tensor_add(h23, h13[:, :, 0:64], h13[:, :, 64:128])
    nc.vector.reduce_sum(Z[:], h23, axis=mybir.AxisListType.X)


def build_program(num_iter: int):
    nc = bacc.Bacc(
        "TRN2", target_bir_lowering=False, debug=False, num_devices=N_CORES
    )
    eT_d = nc.dram_tensor("eT", [128, NL], BF, kind="ExternalInput")
    eaug_d = nc.dram_tensor("eaug", [NL, D + 1], BF, kind="ExternalInput")
    munT0_d = nc.dram_tensor("munT0", [128, K], BF, kind="ExternalInput")
    r_d = nc.dram_tensor("r", [NL, K], FP, kind="ExternalOutput")
    mu_d = nc.dram_tensor("mu", [K, D], FP, kind="ExternalOutput")

    n_upd = num_iter + 1
    with tile.TileContext(nc) as tc:
        with (
            tc.tile_pool(name="persist", bufs=1) as pp,
            tc.tile_pool(name="sb", bufs=3) as sb,
            tc.tile_pool(name="ehp", bufs=8) as ehp,
            tc.tile_pool(name="dram", bufs=1, space="DRAM") as dram,
        ):
            eT_s = pp.tile([128, NL], BF)
            eaug_s = pp.tile([128, T * (D + 1)], BF)
            munT_s = pp.tile([128, K], BF)
            ident = pp.tile([128, 128], BF)

            # ---- prologue: load everything resident ----
            for c in range(16):
                sl = slice(c * 2048, (c + 1) * 2048)
                nc.sync.dma_start(eT_s[:, sl], eT_d.ap()[:, sl])
            for c in range(32):
                src = eaug_d.ap()[c * 1024:(c + 1) * 1024, :].rearrange(
                    "(j p) f -> p j f", p=128
                )
                dst = eaug_s[:, c * 8 * 129:(c + 1) * 8 * 129].rearrange(
                    "p (j f) -> p j f", j=8
                )
                nc.sync.dma_start(dst, src)
            nc.sync.dma_start(munT_s[:], munT0_d.ap())
            make_identity(nc, ident[:])

            # ---- mu-update passes ----
            for it in range(n_upd):
                last = it == n_upd - 1
                with (
                    tc.tile_pool(name=f"pd{it}", bufs=3, space="PSUM") as pdp,
                    tc.tile_pool(name=f"cmp{it}", bufs=1, space="PSUM") as cmp_,
                ):
                    cm0 = cmp_.tile([128, D + 1], FP)
                    cm1 = cmp_.tile([128, D + 1], FP)
                    for c in range(T // CH_U):
                        pd = pdp.tile([128, CH_U * K], FP, tag="pd")
                        for j in range(CH_U):
                            t = c * CH_U + j
                            nc.tensor.matmul(
                                pd[:, j * K:(j + 1) * K],
                                lhsT=eT_s[:, t * 128:(t + 1) * 128],
                                rhs=munT_s[:],
                                start=True,
                                stop=True,
                            )
                        s = sb.tile([128, CH_U * K], BF, tag="s")
                        nc.scalar.activation(s[:], pd[:], AF.Exp, scale=BETA)
                        Z = sb.tile([128, CH_U], FP, tag="z")
                        _zree(nc, sb, s, CH_U, Z)
                        RZ = sb.tile([128, CH_U], FP, tag="rz")
                        nc.vector.reciprocal(RZ[:], Z[:])
                        for j in range(CH_U):
                            t = c * CH_U + j
                            eh = ehp.tile([128, D + 1], BF, tag="eh")
                            nc.gpsimd.tensor_scalar_mul(
                                eh[:],
                                eaug_s[:, t * 129:t * 129 + 129],
                                RZ[:, j:j + 1],
                            )
                            nc.tensor.matmul(
                                cm0[:],
                                lhsT=s[:, j * K:j * K + 128],
                                rhs=eh[:],
                                start=(t == 0),
                                stop=(t == T - 1),
                                skip_group_check=True,
                            )
                            nc.tensor.matmul(
                                cm1[:],
                                lhsT=s[:, j * K + 128:(j + 1) * K],
                                rhs=eh[:],
                                start=(t == 0),
                                stop=(t == T - 1),
                                skip_group_check=True,
                            )
                    # partials -> SBUF -> DRAM bounce for AllReduce
                    cms = sb.tile([128, 2 * (D + 1)], FP, tag="cms")
                    nc.vector.tensor_copy(cms[:, 0:129], cm0[:])
                    nc.vector.tensor_copy(cms[:, 129:258], cm1[:])
                    bin_t = dram.tile([K, D + 1], FP, name=f"arin{it}")
                    nc.sync.dma_start(bin_t[0:128, :], cms[:, 0:129])
                    nc.sync.dma_start(bin_t[128:256, :], cms[:, 129:258])
                bout_t = dram.tile(
                    [K, D + 1], FP, addr_space="Shared", name=f"arout{it}"
                )
                nc.gpsimd.collective_compute(
                    "AllReduce",
                    mybir.AluOpType.add,
                    replica_groups=[list(range(N_CORES))],
                    ins=[bin_t.opt()],
                    outs=[bout_t.opt()],
                )
                # mu_new = cm[:, :128] / cm[:, 128]
                macc = sb.tile([128, 2 * (D + 1)], FP, tag="macc")
                for h in range(2):
                    nc.sync.dma_start(
                        macc[:, h * 129:(h + 1) * 129],
                        bout_t[h * 128:(h + 1) * 128, :],
                    )
                rc = sb.tile([128, 2], FP, tag="rc")
                nc.vector.reciprocal(
                    rc[:, 0:1], macc[:, 128:129]
                )
                nc.vector.reciprocal(
                    rc[:, 1:2], macc[:, 129 + 128:129 + 129]
                )
                mun = sb.tile([128, K], FP, tag="mun")
                for h in range(2):
                    nc.vector.tensor_scalar_mul(
                        mun[:, h * 128:(h + 1) * 128],
                        macc[:, h * 129:h * 129 + 128],
                        rc[:, h:h + 1],
                    )
                if last:
                    for h in range(2):
                        nc.sync.dma_start(
                            mu_d.ap()[h * 128:(h + 1) * 128, :],
                            mun[:, h * 128:(h + 1) * 128],
                        )
                else:
                    # normalize rows: mun *= 1/||mun_row|| (rsqrt via exp/ln)
                    sq = sb.tile([128, K], FP, tag="sq")
                    nc.vector.tensor_mul(sq[:], mun[:], mun[:])
                    n2 = sb.tile([128, 2], FP, tag="n2")
                    sq3 = sq[:].rearrange("p (h d) -> p h d", h=2)
                    nc.vector.reduce_sum(n2[:], sq3, axis=mybir.AxisListType.X)
                    lnn = sb.tile([128, 2], FP, tag="lnn")
                    nc.scalar.activation(lnn[:], n2[:], AF.Ln)
                    rs = sb.tile([128, 2], FP, tag="rs")
                    nc.scalar.activation(rs[:], lnn[:], AF.Exp, scale=-0.5)
                    for h in range(2):
                        nc.vector.tensor_scalar_mul(
                            mun[:, h * 128:(h + 1) * 128],
                            mun[:, h * 128:(h + 1) * 128],
                            rs[:, h:h + 1],
                        )
                # munT_s = mun.T (bf16) via PE transpose
                mnb = sb.tile([128, K], BF, tag="mnb")
                nc.vector.tensor_copy(mnb[:], mun[:])
                with tc.tile_pool(
                    name=f"ptr{it}", bufs=2, space="PSUM"
                ) as ptp:
                    for h in range(2):
                        ptr = ptp.tile([128, 128], BF, tag="ptr")
                        nc.tensor.transpose(
                            ptr[:], mnb[:, h * 128:(h + 1) * 128], ident[:]
                        )
                        nc.vector.tensor_copy(
                            munT_s[:, h * 128:(h + 1) * 128], ptr[:]
                        )

            # ---- final pass: r = exp(beta*dist)/Z ----
            with tc.tile_pool(name="pdf", bufs=2, space="PSUM") as pdf:
                for c in range(T // CH_F):
                    pd = pdf.tile([128, CH_F * K], FP, tag="pdf")
                    for j in range(CH_F):
                        t = c * CH_F + j
                        nc.tensor.matmul(
                            pd[:, j * K:(j + 1) * K],
                            lhsT=eT_s[:, t * 128:(t + 1) * 128],
                            rhs=munT_s[:],
                            start=True,
                            stop=True,
                        )
                    s = sb.tile([128, CH_F * K], BF, tag="sf")
                    nc.scalar.activation(s[:], pd[:], AF.Exp, scale=BETA)
                    Z = sb.tile([128, CH_F], FP, tag="zf")
                    _zree(nc, sb, s, CH_F, Z)
                    RZ = sb.tile([128, CH_F], FP, tag="rzf")
                    nc.vector.reciprocal(RZ[:], Z[:])
                    ro = sb.tile([128, CH_F * K], FP, tag="ro")
                    for j in range(CH_F):
                        osl = ro[:, j * K:(j + 1) * K]
                        isl = s[:, j * K:(j + 1) * K]
                        zsl = RZ[:, j:j + 1]
                        if j < 2:
                            nc.vector.tensor_scalar_mul(osl, isl, zsl)
                        elif j == 2:
                            nc.scalar.activation(
                                osl, isl, AF.Copy, scale=zsl
                            )
                        else:
                            nc.gpsimd.tensor_scalar_mul(osl, isl, zsl)
                    dst = r_d.ap()[
                        c * CH_F * 128:(c + 1) * CH_F * 128, :
                    ].rearrange("(j p) k -> p j k", p=128)
                    nc.sync.dma_start(
                        dst, ro[:].rearrange("p (j k) -> p j k", j=CH_F)
                    )
    nc.compile()
    return nc


def _prep_inputs(x, init_mu):
    x = np.asarray(x, dtype=np.float32)
    mu0 = np.asarray(init_mu, dtype=np.float32)
    e = x / np.linalg.norm(x, axis=1, keepdims=True)
    e = e / np.linalg.norm(e, axis=1, keepdims=True)
    mun0 = mu0 / np.linalg.norm(mu0, axis=1, keepdims=True)
    munT0 = np.ascontiguousarray(mun0.T).astype(ml_dtypes.bfloat16)
    eT = np.ascontiguousarray(e.T.astype(ml_dtypes.bfloat16))
    eaug = np.ones((N, D + 1), dtype=ml_dtypes.bfloat16)
    eaug[:, :D] = e.astype(ml_dtypes.bfloat16)
    in_maps = []
    for c in range(N_CORES):
        sl = slice(c * NL, (c + 1) * NL)
        in_maps.append(
            {
                "eT": np.ascontiguousarray(eT[:, sl]),
                "eaug": np.ascontiguousarray(eaug[sl]),
                "munT0": munT0,
            }
        )
    return in_maps


def _run(x, init_mu, num_iter, **kwargs):
    num_iter = int(np.asarray(num_iter))
    if num_iter not in _cache:
        _cache[num_iter] = build_program(num_iter)
    nc = _cache[num_iter]
    in_maps = _prep_inputs(x, init_mu)
    res = run_bass_kernel_spmd(
        nc, in_maps, core_ids=list(range(N_CORES)), **kwargs
    )
    mu = res.results[0]["mu"]
    r = np.concatenate(
        [res.results[c]["r"] for c in range(N_CORES)], axis=0
    )
    return (mu, r), res


def kernel(x, init_mu, num_iter):
    (mu, r), _ = _run(x, init_mu, num_iter)
    return mu, r


# revision 27
# speedup vs baseline: 2.9503x; 2.9503x over previous
# VQ codebook (soft k-means) kernel for Trainium2, 8 NeuronCores SPMD.
#
# Math (reference): e = l2norm(x rows); repeat num_iter+1 times:
#   dist = e @ l2norm(mu).T ; r = softmax(beta*dist, axis=1)
#   mu <- (r.T @ e) / r.sum(0)[:, None]
# then final r = softmax(beta * e @ mu.T) with UN-normalized mu.
# Returns (mu, r).
#
# Distribution: rows of x (n axis) sharded over 8 cores; mu replicated;
# the (k, d+1) partial sums [r.T @ e | r.sum(0)] AllReduce'd each pass.
#
# Device layout per core (n_local = 32768 rows, 256 row-tiles of 128):
#   eT   (128=d, n_local) bf16   - lhsT for dist matmuls (host pre-transposed)
#   eaug (n_local, 129)   bf16   - [e | 1] rows, rhs for cluster-mean matmuls
#   dist tile = matmul(lhsT=eT_tile, rhs=munT) -> psum (128=n, 256=k)
#   s = exp(beta*dist) on ACT; Z = row-sums via DVE add-tree; on GPSIMD
#   ehat = eaug_tile * (1/Z) then psum_cm[k_half,129] += s_half.T @ ehat on PE.

import numpy as np
import ml_dtypes

import concourse.bass as bass
import concourse.mybir as mybir
import concourse.tile as tile
from concourse import bacc
from concourse.bass_utils import run_bass_kernel_spmd
from concourse.masks import make_identity

N_CORES = 8
N, D, K = 262144, 128, 256
NL = N // N_CORES          # 32768 rows per core
T = NL // 128              # 256 row-tiles per core
BETA = 5.0
FP = mybir.dt.float32
BF = mybir.dt.bfloat16
AF = mybir.ActivationFunctionType
CH_U = 4                   # row-tiles per chunk in mu-update passes
CH_F = 8                   # row-tiles per chunk in final pass

_cache = {}


def _zree(nc, sb, s, nt, Z, s1_gp=False):
    """Row-sums of s (128, nt, 256) -> Z (128, nt) fp32 via bf16 add tree.
    s1_gp routes the first (largest) stage to GPSIMD for load balance."""
    h1 = sb.tile([128, nt * 128], BF, tag="zh1")
    s3 = s[:].rearrange("p (j k) -> p j k", j=nt)
    h13 = h1[:].rearrange("p (j k) -> p j k", j=nt)
    e1 = nc.gpsimd if s1_gp else nc.vector
    e1.tensor_add(h13, s3[:, :, 0:128], s3[:, :, 128:256])
    h2 = sb.tile([128, nt * 64], BF, tag="zh2")
    h23 = h2[:].rearrange("p (j k) -> p j k", j=nt)
    nc.vector.tensor_add(h23, h13[:, :, 0:64], h13[:, :, 64:128])
    nc.vector.reduce_sum(Z[:], h23, axis=mybir.AxisListType.X)


def build_program(num_iter: int, single: bool = False):
    """single=True builds a 1-core variant with the AllReduce replaced by a
    DRAM->DRAM copy, for TimelineSim-based performance analysis."""
    nc = bacc.Bacc(
        "TRN2",
        target_bir_lowering=False,
        debug=False,
        num_devices=1 if single else N_CORES,
    )
    eT_d = nc.dram_tensor("eT", [128, NL], BF, kind="ExternalInput")
    # eaug pre-tiled on host to SBUF layout: [p, t*129+f] = [e|1][t*128+p, f]
    eaug_d = nc.dram_tensor(
        "eaug", [128, T * (D + 1)], BF, kind="ExternalInput"
    )
    munT0_d = nc.dram_tensor("munT0", [128, K], BF, kind="ExternalInput")
    r_d = nc.dram_tensor("r", [NL, K], FP, kind="ExternalOutput")
    mu_d = nc.dram_tensor("mu", [K, D], FP, kind="ExternalOutput")

    n_upd = num_iter + 1
    with tile.TileContext(nc) as tc:
        with (
            tc.tile_pool(name="persist", bufs=1) as pp,
            tc.tile_pool(name="sbu", bufs=8) as sbu,
            tc.tile_pool(name="sbb", bufs=2) as sbb,
            tc.tile_pool(name="sbf", bufs=2) as sbf,
            tc.tile_pool(name="ehp", bufs=24) as ehp,
            tc.tile_pool(name="dram", bufs=1, space="DRAM") as dram,
        ):
            eT_s = pp.tile([128, NL], BF)
            eaug_s = pp.tile([128, T * (D + 1)], BF)
            munT_s = pp.tile([128, K], BF)
            ident_f = pp.tile([128, 128], FP)

            # ---- prologue: load everything resident (first-needed first) ----
            nc.sync.dma_start(munT_s[:], munT0_d.ap())
            make_identity(nc, ident_f[:])
            for c in range(16):
                sl = slice(c * 2048, (c + 1) * 2048)
                nc.sync.dma_start(eT_s[:, sl], eT_d.ap()[:, sl])
                sl2 = slice(c * 2064, (c + 1) * 2064)
                nc.sync.dma_start(eaug_s[:, sl2], eaug_d.ap()[:, sl2])

            # ---- mu-update passes ----
            for it in range(n_upd):
                last = it == n_upd - 1
                with (
                    tc.tile_pool(name=f"pd{it}", bufs=3, space="PSUM") as pdp,
                    tc.tile_pool(name=f"cmp{it}", bufs=1, space="PSUM") as cmp_,
                ):
                    cm0 = cmp_.tile([128, D + 1], FP)
                    cm1 = cmp_.tile([128, D + 1], FP)
                    for c in range(T // CH_U):
                        pd = pdp.tile([128, CH_U * K], FP, tag="pd")
                        for j in range(CH_U):
                            t = c * CH_U + j
                            nc.tensor.matmul(
                                pd[:, j * K:(j + 1) * K],
                                lhsT=eT_s[:, t * 128:(t + 1) * 128],
                                rhs=munT_s[:],
                                start=True,
                                stop=True,
                            )
                        s = sbu.tile([128, CH_U * K], BF, tag="s")
                        nc.scalar.activation(s[:], pd[:], AF.Exp, scale=BETA)
                        Z = sbu.tile([128, CH_U], FP, tag="z")
                        _zree(nc, sbu, s, CH_U, Z, s1_gp=(c % 2 == 0))
                        RZ = sbu.tile([128, CH_U], FP, tag="rz")
                        nc.vector.reciprocal(RZ[:], Z[:])
                        for j in range(CH_U):
                            t = c * CH_U + j
                            eh = ehp.tile([128, D + 1], BF, tag="eh")
                            eng = (
                                nc.gpsimd
                                if (c % 2 == 1 and j < 2)
                                else nc.vector
                            )
                            eng.tensor_scalar_mul(
                                eh[:],
                                eaug_s[:, t * 129:t * 129 + 129],
                                RZ[:, j:j + 1],
                            )
                            nc.tensor.matmul(
                                cm0[:],
                                lhsT=s[:, j * K:j * K + 128],
                                rhs=eh[:],
                                start=(t == 0),
                                stop=(t == T - 1),
                                skip_group_check=True,
                            )
                            nc.tensor.matmul(
                                cm1[:],
                                lhsT=s[:, j * K + 128:(j + 1) * K],
                                rhs=eh[:],
                                start=(t == 0),
                                stop=(t == T - 1),
                                skip_group_check=True,
                            )
                    # partials -> SBUF -> DRAM bounce for AllReduce
                    cms = sbb.tile([128, 2 * (D + 1)], FP, tag="cms")
                    nc.vector.tensor_copy(cms[:, 0:129], cm0[:])
                    nc.vector.tensor_copy(cms[:, 129:258], cm1[:])
                    bin_t = dram.tile([K, D + 1], FP, name=f"arin{it}")
                    nc.sync.dma_start(bin_t[0:128, :], cms[:, 0:129])
                    nc.sync.dma_start(bin_t[128:256, :], cms[:, 129:258])
                bout_t = dram.tile(
                    [K, D + 1], FP, addr_space="Shared", name=f"arout{it}"
                )
                if single:
                    nc.sync.dma_start(bout_t[:], bin_t[:])
                else:
                    nc.gpsimd.collective_compute(
                        "AllReduce",
                        mybir.AluOpType.add,
                        replica_groups=[list(range(N_CORES))],
                        ins=[bin_t.opt()],
                        outs=[bout_t.opt()],
                    )
                # mu_new = cm[:, :128] / cm[:, 128]
                macc = sbb.tile([128, 2 * (D + 1)], FP, tag="macc")
                for h in range(2):
                    nc.sync.dma_start(
                        macc[:, h * 129:(h + 1) * 129],
                        bout_t[h * 128:(h + 1) * 128, :],
                    )
                rc = sbb.tile([128, 2], FP, tag="rc")
                nc.vector.reciprocal(
                    rc[:, 0:1], macc[:, 128:129]
                )
                nc.vector.reciprocal(
                    rc[:, 1:2], macc[:, 129 + 128:129 + 129]
                )
                mun = sbb.tile([128, K], FP, tag="mun")
                for h in range(2):
                    nc.vector.tensor_scalar_mul(
                        mun[:, h * 128:(h + 1) * 128],
                        macc[:, h * 129:h * 129 + 128],
                        rc[:, h:h + 1],
                    )
                if last:
                    for h in range(2):
                        nc.sync.dma_start(
                            mu_d.ap()[h * 128:(h + 1) * 128, :],
                            mun[:, h * 128:(h + 1) * 128],
                        )
                else:
                    # normalize rows: mun *= 1/||mun_row|| (rsqrt via exp/ln)
                    sq = sbb.tile([128, K], FP, tag="sq")
                    nc.vector.tensor_mul(sq[:], mun[:], mun[:])
                    n2 = sbb.tile([128, 2], FP, tag="n2")
                    sq3 = sq[:].rearrange("p (h d) -> p h d", h=2)
                    nc.vector.reduce_sum(n2[:], sq3, axis=mybir.AxisListType.X)
                    lnn = sbb.tile([128, 2], FP, tag="lnn")
                    nc.scalar.activation(lnn[:], n2[:], AF.Ln)
                    rs = sbb.tile([128, 2], FP, tag="rs")
                    nc.scalar.activation(rs[:], lnn[:], AF.Exp, scale=-0.5)
                    for h in range(2):
                        nc.vector.tensor_scalar_mul(
                            mun[:, h * 128:(h + 1) * 128],
                            mun[:, h * 128:(h + 1) * 128],
                            rs[:, h:h + 1],
                        )
                # munT_s = mun.T (bf16) via PE transpose (fp32 in, 2cyc/row)
                with tc.tile_pool(
                    name=f"ptr{it}", bufs=2, space="PSUM"
                ) as ptp:
                    for h in range(2):
                        ptr = ptp.tile([128, 128], FP, tag="ptr")
                        nc.tensor.transpose(
                            ptr[:], mun[:, h * 128:(h + 1) * 128], ident_f[:]
                        )
                        nc.vector.tensor_copy(
                            munT_s[:, h * 128:(h + 1) * 128], ptr[:]
                        )

            # ---- final pass: r = exp(beta*dist)/Z ----
            with tc.tile_pool(name="pdf", bufs=2, space="PSUM") as pdf:
                for c in range(T // CH_F):
                    pd = pdf.tile([128, CH_F * K], FP, tag="pdf")
                    for j in range(CH_F):
                        t = c * CH_F + j
                        nc.tensor.matmul(
                            pd[:, j * K:(j + 1) * K],
                            lhsT=eT_s[:, t * 128:(t + 1) * 128],
                            rhs=munT_s[:],
                            start=True,
                            stop=True,
                        )
                    s = sbf.tile([128, CH_F * K], BF, tag="sf")
                    nc.scalar.activation(s[:], pd[:], AF.Exp, scale=BETA)
                    Z = sbf.tile([128, CH_F], FP, tag="zf")
                    _zree(nc, sbf, s, CH_F, Z)
                    RZ = sbf.tile([128, CH_F], FP, tag="rzf")
                    nc.vector.reciprocal(RZ[:], Z[:])
                    ro = sbf.tile([128, CH_F * K], FP, tag="ro")
                    for j in range(CH_F):
                        osl = ro[:, j * K:(j + 1) * K]
                        isl = s[:, j * K:(j + 1) * K]
                        zsl = RZ[:, j:j + 1]
                        if j < 2:
                            nc.vector.tensor_scalar_mul(osl, isl, zsl)
                        elif j < 4:
                            nc.scalar.activation(
                                osl, isl, AF.Copy, scale=zsl
                            )
                        else:
                            nc.gpsimd.tensor_scalar_mul(osl, isl, zsl)
                    dst = r_d.ap()[
                        c * CH_F * 128:(c + 1) * CH_F * 128, :
                    ].rearrange("(j p) k -> p j k", p=128)
                    nc.sync.dma_start(
                        dst, ro[:].rearrange("p (j k) -> p j k", j=CH_F)
                    )
    nc.compile()
    return nc


def _prep_inputs(x, init_mu):
    x = np.asarray(x, dtype=np.float32)
    mu0 = np.asarray(init_mu, dtype=np.float32)
    e = x / np.linalg.norm(x, axis=1, keepdims=True)
    e = e / np.linalg.norm(e, axis=1, keepdims=True)
    mun0 = mu0 / np.linalg.norm(mu0, axis=1, keepdims=True)
    munT0 = np.ascontiguousarray(mun0.T).astype(ml_dtypes.bfloat16)
    eT = np.ascontiguousarray(e.T.astype(ml_dtypes.bfloat16))
    eaug = np.ones((N, D + 1), dtype=ml_dtypes.bfloat16)
    eaug[:, :D] = e.astype(ml_dtypes.bfloat16)
    in_maps = []
    for c in range(N_CORES):
        sl = slice(c * NL, (c + 1) * NL)
        # pre-tile eaug shard to SBUF layout (128, T*129)
        ea = (
            eaug[sl]
            .reshape(T, 128, D + 1)
            .transpose(1, 0, 2)
            .reshape(128, T * (D + 1))
        )
        in_maps.append(
            {
                "eT": np.ascontiguousarray(eT[:, sl]),
                "eaug": np.ascontiguousarray(ea),
                "munT0": munT0,
            }
        )
    return in_maps


def _run(x, init_mu, num_iter, **kwargs):
    num_iter = int(np.asarray(num_iter))
    if num_iter not in _cache:
        _cache[num_iter] = build_program(num_iter)
    nc = _cache[num_iter]
    in_maps = _prep_inputs(x, init_mu)
    res = run_bass_kernel_spmd(
        nc, in_maps, core_ids=list(range(N_CORES)), **kwargs
    )
    mu = res.results[0]["mu"]
    r = np.concatenate(
        [res.results[c]["r"] for c in range(N_CORES)], axis=0
    )
    return (mu, r), res


def kernel(x, init_mu, num_iter):
    (mu, r), _ = _run(x, init_mu, num_iter)
    return mu, r
